# revision 14
# baseline (speedup 1.0000x reference)
"""Bass/Trainium2 kernel for nn_ExpMovAvgModel (sparse_attention).

Math (per batch row b, query t, key s, H=128 hidden):
    x      = embd[seq]                        # [T, H] gathered rows
    xhat   = x / |x|                          # row-normalized
    raw    = xhat @ xhat.T                    # cosine similarity [T, T]
    sim01  = 0.5*(raw+1) masked to s < t
    delta  = reversed-cumsum_s(sim01)         # suffix sums over keys
    lam    = exp(x @ lam_w + lam_b)
    w      = sim01 * exp(-lam*delta)
    yhat   = clip((w @ y) / (sum_s w + 1e-6), 0.01, 0.99)

Restructure (v5): with d[s] = raw[t,s]+1 = 2*sim01 (masked to 0 at s>=t):

    suf[s]  = sum_{k>=s} d[k]   = 2*delta[t,s]   (mask freezes it at k=t-1)
    E[s]    = exp(-lam/2 * suf[s])
    den*2   = sum_s d[s] * E[s]
    num*2   = sum_s d[s] * E[s] * y[s]

ONE reversed cumsum scan (negative-stride APs on DVE) replaces the two
fused multiply-add scans of the original kernel, and the num/den
reductions leave the DVE entirely: Wf = d*E is a 2x-mode fp16
tensor_tensor, each 128-col block of Wf is DMA-TRANSPOSED (idle DMA
queues), and one PE matmul per block with rhs = [ones | y-col] yields
both sums, accumulated over blocks in PSUM fp32.  Per-block engine
split (measured ns/col):

    PE : raw = xhatT_tb^T @ xhatT[:, :W]  -> PSUM fp32      (~1.07)
    ACT: d16 = Copy(raw + 1)              -> fp16 SBUF      (~1.31)
    DVE: mask diag block: d16 = min(d16, mask16) (0 at s>=t)
    DVE: suf = reverse-cumsum(d16)        -> fp32 SBUF      (~2.2)
    ACT: E   = Exp(nhl*suf + 10*ln2)      -> fp16 SBUF      (~1.55)
    DVE: Wf  = d16*E (2x mode)            -> fp16 SBUF      (~0.56)
    DMA: WfT_kb = transpose(Wf[:, kb])    -> fp16 SBUF
    PE : redps[:, 2tb:2tb+2] += WfT_kb @ [ones | y_kb]      (accum)

The 2^10 prescale on E (bias = 10*ln2) lifts small-E rows out of
fp16's subnormal range; the scale cancels in num/den (den offset
adjusted).  Software pipeline over tb keeps every queue busy; the
reduce matmuls for tb run two iterations behind the scan.

Sharding: data-parallel over batch B=32 -> 4 batches per core x 8 cores.
"""

import os
import sys

import numpy as np

for _p in ("/opt/trn_rl_repo",):
    if _p not in sys.path and os.path.isdir(_p):
        sys.path.append(_p)

import concourse.bass as bass
import concourse.tile as tile
from concourse import bacc, mybir

P = 128            # partitions / hidden dim
T = 1024           # sequence length
NJ = T // P        # 8 column-blocks
NB_PER_CORE = 4    # batches per core
N_CORES = 8

F32 = mybir.dt.float32
F16 = mybir.dt.float16
BF16 = mybir.dt.bfloat16


def build_program():
    nc = bacc.Bacc(
        "TRN2",
        target_bir_lowering=False,
        debug=False,
        num_devices=N_CORES,
    )

    xh = nc.dram_tensor("xh", [NB_PER_CORE, P, T], F16, kind="ExternalInput").ap()
    nhl = nc.dram_tensor("nhl", [NB_PER_CORE, P, NJ], F32, kind="ExternalInput").ap()
    yok = nc.dram_tensor("yok", [NB_PER_CORE, P, 2 * NJ], F16,
                         kind="ExternalInput").ap()
    mask16 = nc.dram_tensor("mask16", [P, P], F16, kind="ExternalInput").ap()
    out = nc.dram_tensor("out", [NB_PER_CORE, P, NJ], F32, kind="ExternalOutput").ap()

    with tile.TileContext(nc) as tc:
        _build_body(tc, xh, nhl, yok, mask16, out)

    nc.compile()
    return nc


def _build_body(tc, xh, nhl, yok, mask16, out):
    from contextlib import ExitStack

    nc = tc.nc
    Exp = mybir.ActivationFunctionType.Exp
    Copy = mybir.ActivationFunctionType.Copy
    ADD = mybir.AluOpType.add
    MULT = mybir.AluOpType.mult
    MAX = mybir.AluOpType.max
    MIN = mybir.AluOpType.min

    with ExitStack() as ctx:
        pconst = ctx.enter_context(tc.tile_pool(name="pconst", bufs=1))
        pxt = ctx.enter_context(tc.tile_pool(name="pxt", bufs=2))
        pwork = ctx.enter_context(tc.tile_pool(name="pwork", bufs=3))
        psmall = ctx.enter_context(tc.tile_pool(name="psmall", bufs=2))
        pps = ctx.enter_context(tc.tile_pool(name="pps", bufs=3, space="PSUM"))
        ppr = ctx.enter_context(tc.tile_pool(name="ppr", bufs=2, space="PSUM"))

        def prep_dma(b):
            """Issue batch b's input DMAs - pure DMA traffic, issued one
            batch ahead at the previous batch's tb=0.  Batch 0 splits the
            xh DMA so the first matmul waits only on its first block."""
            xhatT = pxt.tile([P, T], F16, tag="xhatT")
            if b == 0:
                nc.sync.dma_start(out=xhatT[:, 0:P], in_=xh[0][:, 0:P])
                nc.sync.dma_start(out=xhatT[:, P:T], in_=xh[0][:, P:T])
            else:
                nc.sync.dma_start(out=xhatT[:], in_=xh[b])
            nhl_sb = pxt.tile([P, NJ], F32, tag="nhl")
            nc.sync.dma_start(out=nhl_sb[:], in_=nhl[b])
            yok_sb = pxt.tile([P, 2 * NJ], F16, tag="yok")
            nc.sync.dma_start(out=yok_sb[:], in_=yok[b])
            return xhatT, nhl_sb, yok_sb

        nxt = prep_dma(0)
        mask_sb = pconst.tile([P, P], F16)
        nc.sync.dma_start(out=mask_sb[:], in_=mask16)
        zeros16 = pconst.tile([P, T], F16)
        nc.vector.memset(zeros16[:], 0.0)
        # Exp bias = 10*ln2 (E prescale by 2^10, see exp_act)
        ebias = pconst.tile([P, 1], F32)
        nc.vector.memset(ebias[:], 6.931471805599453)

        for b in range(NB_PER_CORE):
            xhatT, nhl_sb, yok_sb = nxt

            # den/num accumulators: cols [2*tb, 2*tb+1] per query block
            redps = ppr.tile([P, 2 * NJ], F32, tag="redps")

            def matmul(tb):
                W = (tb + 1) * P
                Woff = W - P
                raw = pps.tile([P, T], F32, tag="raw")
                for h in range((W + 511) // 512):
                    w0 = h * 512
                    wh = min(W, w0 + 512) - w0
                    nc.tensor.matmul(
                        out=raw[:, w0 : w0 + wh],
                        lhsT=xhatT[:, Woff:W],
                        rhs=xhatT[:, w0 : w0 + wh],
                        start=True,
                        stop=True,
                    )
                return raw

            def d16_copy(tb, raw):
                # d16 = raw + 1 = 2*sim01 (unmasked), fp16; frees PSUM
                W = (tb + 1) * P
                d16 = pwork.tile([P, T], F16, tag="d16")
                nc.scalar.activation(
                    out=d16[:, :W], in_=raw[:, :W], func=Copy, bias=1.0, scale=1.0
                )
                return d16

            def mask(tb, d16):
                # strict-causal: d16 -> 0 where s >= t inside the diag block
                W = (tb + 1) * P
                nc.vector.tensor_tensor(
                    out=d16[:, W - P : W], in0=d16[:, W - P : W], in1=mask_sb[:],
                    op=MIN,
                )

            def scan(tb, d16):
                # suf[s] = sum_{k>=s} d16[k]  (reverse cumsum, frozen at t-1)
                W = (tb + 1) * P
                suf = pwork.tile([P, T], F32, tag="suf")
                nc.vector.tensor_tensor_scan(
                    out=suf[:, W - 1 :: -1],
                    data0=d16[:, W - 1 :: -1],
                    data1=zeros16[:, :W],
                    initial=0.0,
                    op0=ADD,
                    op1=ADD,
                )
                return suf

            def exp_act(tb, suf):
                W = (tb + 1) * P
                nhl_c = nhl_sb[:, tb : tb + 1]
                E = pwork.tile([P, T], F16, tag="E")
                nc.scalar.activation(
                    out=E[:, :W], in_=suf[:, :W], func=Exp,
                    bias=ebias[:], scale=nhl_c
                )
                return E

            def tt1(tb, d16, E):
                # Wf = d16*E, all-fp16 SBUF -> DVE 2x mode
                W = (tb + 1) * P
                Wf = pwork.tile([P, T], F16, tag="Wf")
                nc.vector.tensor_tensor(
                    out=Wf[:, :W], in0=d16[:, :W], in1=E[:, :W], op=MULT
                )
                return Wf

            def transpose_wf(tb, Wf):
                W = (tb + 1) * P
                WfT = pwork.tile([P, T], F16, tag="WfT")
                for kb in range(tb + 1):
                    nc.sync.dma_start_transpose(
                        out=WfT[:, kb * P : (kb + 1) * P],
                        in_=Wf[:, kb * P : (kb + 1) * P],
                    )
                return WfT

            def reduce_mm(tb, WfT):
                # den/num: [WfT_kb]^T @ [ones | y_kb], accumulated over kb
                for kb in range(tb + 1):
                    nc.tensor.matmul(
                        out=redps[:, 2 * tb : 2 * tb + 2],
                        lhsT=WfT[:, kb * P : (kb + 1) * P],
                        rhs=yok_sb[:, 2 * kb : 2 * kb + 2],
                        start=(kb == 0),
                        stop=(kb == tb),
                    )

            # software pipeline over tb:
            #   PE : matmul(i+2), reduce_mm(i-2)
            #   DVE: scan(i), tt1(i-1), mask(i+1)
            #   ACT: d16(i+1), E(i)
            #   DMA: transpose_wf(i-1)
            raws, d16s, Es, sufs, Wfs, WfTs = {}, {}, {}, {}, {}, {}
            raws[0] = matmul(0)
            d16s[0] = d16_copy(0, raws[0])
            mask(0, d16s[0])
            raws[1] = matmul(1)
            for i in range(NJ + 2):
                if i == 0 and b + 1 < NB_PER_CORE:
                    nxt = prep_dma(b + 1)
                if i < NJ:
                    sufs[i] = scan(i, d16s[i])
                    Es[i] = exp_act(i, sufs[i])
                if i + 2 <= NJ - 1:
                    raws[i + 2] = matmul(i + 2)
                if i + 1 <= NJ - 1:
                    d16s[i + 1] = d16_copy(i + 1, raws[i + 1])
                    mask(i + 1, d16s[i + 1])
                if 0 <= i - 1 < NJ:
                    Wfs[i - 1] = tt1(i - 1, d16s[i - 1], Es[i - 1])
                    WfTs[i - 1] = transpose_wf(i - 1, Wfs[i - 1])
                if 0 <= i - 2 < NJ:
                    reduce_mm(i - 2, WfTs[i - 2])

            # ---- finalize (den at even cols, num at odd cols) ----
            wse = psmall.tile([P, NJ], F32, tag="wse")
            nc.vector.tensor_scalar(
                out=wse[:], in0=redps[:, 0 : 2 * NJ : 2], scalar1=2e-6 * 1024.0,
                scalar2=None, op0=ADD
            )
            rcp = psmall.tile([P, NJ], F32, tag="rcp")
            nc.vector.reciprocal(out=rcp[:], in_=wse[:])
            yh = psmall.tile([P, NJ], F32, tag="yh")
            nc.vector.tensor_tensor(
                out=yh[:], in0=redps[:, 1 : 2 * NJ : 2], in1=rcp[:], op=MULT
            )
            yc = psmall.tile([P, NJ], F32, tag="yc")
            nc.vector.tensor_scalar(
                out=yc[:], in0=yh[:], scalar1=0.01, scalar2=0.99, op0=MAX, op1=MIN
            )
            nc.sync.dma_start(out=out[b], in_=yc[:])


def shard_inputs(y, problem_seq, embd_weight, lam_w, lam_b):
    """Build per-core input maps (host-side layout prep, not device time)."""
    B = y.shape[0]
    assert B == N_CORES * NB_PER_CORE
    seq = np.ascontiguousarray(problem_seq).astype(np.int64)
    yf = np.ascontiguousarray(y).astype(np.float32)
    emb = np.ascontiguousarray(embd_weight).astype(np.float32)
    lamw = np.asarray(lam_w, dtype=np.float32).reshape(P, 1)
    lamb = np.float32(np.asarray(lam_b).reshape(-1)[0])

    norm = np.linalg.norm(emb, axis=1, keepdims=True)
    xhat16 = (emb / norm).astype(np.float16)           # [V, H]
    nhl32 = (-0.5 * np.exp(emb @ lamw + lamb)).astype(np.float32)[:, 0]  # [V]

    colv, rowv = np.meshgrid(np.arange(P), np.arange(P))
    # min-mask on d16 = raw+1: pass below diagonal, clamp to 0 at/above
    mask16 = np.where(colv < rowv, np.float16(65504.0), np.float16(0.0)).astype(
        np.float16
    )

    in_maps = []
    for c in range(N_CORES):
        sl = slice(c * NB_PER_CORE, (c + 1) * NB_PER_CORE)
        seq_c = seq[sl]                                 # [NB, T]
        # xh[b, h, t] = xhat16[seq[b, t], h]
        xh = np.ascontiguousarray(
            xhat16[seq_c].transpose(0, 2, 1)            # [NB, H, T]
        )
        # nhl[b, p, tb] = -lam/2 of token tb*128+p
        nhl_c = np.ascontiguousarray(
            nhl32[seq_c].reshape(NB_PER_CORE, NJ, P).transpose(0, 2, 1)
        )
        # yok[b, k, 2*kb] = 1, yok[b, k, 2*kb+1] = y[b, kb*128+k]
        yok_c = np.zeros((NB_PER_CORE, P, 2 * NJ), np.float16)
        yok_c[:, :, 0::2] = 1.0
        yok_c[:, :, 1::2] = (
            yf[sl].reshape(NB_PER_CORE, NJ, P).transpose(0, 2, 1).astype(np.float16)
        )
        in_maps.append(
            {
                "xh": xh,
                "nhl": nhl_c,
                "yok": np.ascontiguousarray(yok_c),
                "mask16": mask16,
            }
        )
    return in_maps


def unshard_output(results):
    """results: list of 8 dicts with 'out' [4, 128, 8] -> yhat [32, 1024]."""
    parts = []
    for c in range(N_CORES):
        o = results[c]["out"]  # [NB, P, NJ]; yhat[b, j*128+p] = o[b, p, j]
        parts.append(o.transpose(0, 2, 1).reshape(NB_PER_CORE, T))
    return np.concatenate(parts, axis=0).astype(np.float32)


_NC_CACHE = None


def _get_program():
    global _NC_CACHE
    if _NC_CACHE is None:
        _NC_CACHE = build_program()
    return _NC_CACHE


def kernel(y, problem_seq, embd_weight, lam_w, lam_b, _trace=False, **trace_kwargs):
    from concourse.bass_utils import run_bass_kernel_spmd

    nc = _get_program()
    in_maps = shard_inputs(y, problem_seq, embd_weight, lam_w, lam_b)
    res = run_bass_kernel_spmd(
        nc, in_maps, core_ids=list(range(N_CORES)), trace=_trace, **trace_kwargs
    )
    outp = unshard_output(res.results)
    if _trace:
        return outp, res
    return outp


if __name__ == "__main__":
    rng = np.random.default_rng(0)
    y = rng.random((32, T), dtype=np.float32)
    seq = rng.integers(0, 50000, size=(32, T)).astype(np.int32)
    emb = rng.standard_normal((50000, P), dtype=np.float32)
    lw = (rng.standard_normal((P, 1), dtype=np.float32) / np.sqrt(P)).astype(np.float32)
    lb = (rng.standard_normal((1,), dtype=np.float32) * 0.01).astype(np.float32)
    outp = kernel(y, seq, emb, lw, lb)
    print("out", outp.shape, outp.dtype, outp[:2, :5])


# revision 16
# speedup vs baseline: 2.4211x; 2.4211x over previous
"""Bass/Trainium2 kernel for nn_ExpMovAvgModel (sparse_attention).

Math (per batch row b, query t, key s, H=128 hidden):
    x      = embd[seq]                        # [T, H] gathered rows
    xhat   = x / |x|                          # row-normalized
    raw    = xhat @ xhat.T                    # cosine similarity [T, T]
    sim01  = 0.5*(raw+1) masked to s < t
    delta  = reversed-cumsum_s(sim01)         # suffix sums over keys
    lam    = exp(x @ lam_w + lam_b)
    w      = sim01 * exp(-lam*delta)
    yhat   = clip((w @ y) / (sum_s w + 1e-6), 0.01, 0.99)

Restructure (v5): with d[s] = raw[t,s]+1 = 2*sim01 (masked to 0 at s>=t):

    suf[s]  = sum_{k>=s} d[k]   = 2*delta[t,s]   (mask freezes it at k=t-1)
    E[s]    = exp(-lam/2 * suf[s])
    den*2   = sum_s d[s] * E[s]
    num*2   = sum_s d[s] * E[s] * y[s]

ONE reversed cumsum scan (negative-stride APs on DVE) replaces the two
fused multiply-add scans of the original kernel, and the num/den
reductions leave the DVE entirely: Wf = d*E is a 2x-mode fp16
tensor_tensor, each 128-col block of Wf is DMA-TRANSPOSED (idle DMA
queues), and one PE matmul per block with rhs = [ones | y-col] yields
both sums, accumulated over blocks in PSUM fp32.  Per-block engine
split (measured ns/col):

    PE : raw = xhatT_tb^T @ xhatT[:, :W]  -> PSUM fp32      (~1.07)
    ACT: d16 = Copy(raw + 1)              -> fp16 SBUF      (~1.31)
    DVE: mask diag block: d16 = min(d16, mask16) (0 at s>=t)
    DVE: suf = reverse-cumsum(d16)        -> fp32 SBUF      (~2.2)
    ACT: E   = Exp(nhl*suf + 10*ln2)      -> fp16 SBUF      (~1.55)
    DVE: Wf  = d16*E (2x mode)            -> fp16 SBUF      (~0.56)
    DMA: WfT_kb = transpose(Wf[:, kb])    -> fp16 SBUF
    PE : redps[:, 2tb:2tb+2] += WfT_kb @ [ones | y_kb]      (accum)

The 2^10 prescale on E (bias = 10*ln2) lifts small-E rows out of
fp16's subnormal range; the scale cancels in num/den (den offset
adjusted).  Software pipeline over tb keeps every queue busy; the
reduce matmuls for tb run two iterations behind the scan.

Sharding: data-parallel over batch B=32 -> 4 batches per core x 8 cores.
"""

import os
import sys

import numpy as np

for _p in ("/opt/trn_rl_repo",):
    if _p not in sys.path and os.path.isdir(_p):
        sys.path.append(_p)

import concourse.bass as bass
import concourse.tile as tile
from concourse import bacc, mybir

P = 128            # partitions / hidden dim
T = 1024           # sequence length
NJ = T // P        # 8 column-blocks
NB_PER_CORE = 4    # batches per core
N_CORES = 8

F32 = mybir.dt.float32
F16 = mybir.dt.float16
BF16 = mybir.dt.bfloat16


def build_program():
    nc = bacc.Bacc(
        "TRN2",
        target_bir_lowering=False,
        debug=False,
        num_devices=N_CORES,
    )

    xh = nc.dram_tensor("xh", [NB_PER_CORE, P, T], F16, kind="ExternalInput").ap()
    nhl = nc.dram_tensor("nhl", [NB_PER_CORE, P, NJ], F32, kind="ExternalInput").ap()
    yok = nc.dram_tensor("yok", [NB_PER_CORE, P, 2 * NJ], F16,
                         kind="ExternalInput").ap()
    mask16 = nc.dram_tensor("mask16", [P, P], F16, kind="ExternalInput").ap()
    out = nc.dram_tensor("out", [NB_PER_CORE, P, NJ], F32, kind="ExternalOutput").ap()

    with tile.TileContext(nc) as tc:
        _build_body(tc, xh, nhl, yok, mask16, out)

    nc.compile()
    return nc


def _build_body(tc, xh, nhl, yok, mask16, out):
    from contextlib import ExitStack

    nc = tc.nc
    Exp = mybir.ActivationFunctionType.Exp
    Copy = mybir.ActivationFunctionType.Copy
    ADD = mybir.AluOpType.add
    MULT = mybir.AluOpType.mult
    MAX = mybir.AluOpType.max
    MIN = mybir.AluOpType.min

    with ExitStack() as ctx:
        pconst = ctx.enter_context(tc.tile_pool(name="pconst", bufs=1))
        pxt = ctx.enter_context(tc.tile_pool(name="pxt", bufs=2))
        pwork = ctx.enter_context(tc.tile_pool(name="pwork", bufs=3))
        psmall = ctx.enter_context(tc.tile_pool(name="psmall", bufs=2))
        pps = ctx.enter_context(tc.tile_pool(name="pps", bufs=3, space="PSUM"))
        ppr = ctx.enter_context(tc.tile_pool(name="ppr", bufs=2, space="PSUM"))

        def prep_dma(b):
            """Issue batch b's input DMAs - pure DMA traffic, issued one
            batch ahead at the previous batch's tb=0.  Batch 0 splits the
            xh DMA so the first matmul waits only on its first block."""
            xhatT = pxt.tile([P, T], F16, tag="xhatT")
            if b == 0:
                nc.sync.dma_start(out=xhatT[:, 0:P], in_=xh[0][:, 0:P])
                nc.sync.dma_start(out=xhatT[:, P:T], in_=xh[0][:, P:T])
            else:
                nc.sync.dma_start(out=xhatT[:], in_=xh[b])
            nhl_sb = pxt.tile([P, NJ], F32, tag="nhl")
            nc.sync.dma_start(out=nhl_sb[:], in_=nhl[b])
            yok_sb = pxt.tile([P, 2 * NJ], F16, tag="yok")
            nc.sync.dma_start(out=yok_sb[:], in_=yok[b])
            return xhatT, nhl_sb, yok_sb

        nxt = prep_dma(0)
        mask_sb = pconst.tile([P, P], F16)
        nc.sync.dma_start(out=mask_sb[:], in_=mask16)
        zeros16 = pconst.tile([P, T], F16)
        nc.vector.memset(zeros16[:], 0.0)
        # Exp bias = 10*ln2 (E prescale by 2^10, see exp_act)
        ebias = pconst.tile([P, 1], F32)
        nc.vector.memset(ebias[:], 6.931471805599453)

        for b in range(NB_PER_CORE):
            xhatT, nhl_sb, yok_sb = nxt

            # den/num accumulators: cols [2*tb, 2*tb+1] per query block
            redps = ppr.tile([P, 2 * NJ], F32, tag="redps")

            def matmul(tb):
                W = (tb + 1) * P
                Woff = W - P
                raw = pps.tile([P, T], F32, tag="raw")
                for h in range((W + 511) // 512):
                    w0 = h * 512
                    wh = min(W, w0 + 512) - w0
                    nc.tensor.matmul(
                        out=raw[:, w0 : w0 + wh],
                        lhsT=xhatT[:, Woff:W],
                        rhs=xhatT[:, w0 : w0 + wh],
                        start=True,
                        stop=True,
                    )
                return raw

            def d16_copy(tb, raw):
                # d16 = raw + 1 = 2*sim01 (unmasked), fp16; frees PSUM
                W = (tb + 1) * P
                d16 = pwork.tile([P, T], F16, tag="d16")
                nc.scalar.activation(
                    out=d16[:, :W], in_=raw[:, :W], func=Copy, bias=1.0, scale=1.0
                )
                return d16

            def mask(tb, d16):
                # strict-causal: d16 -> 0 where s >= t inside the diag block
                W = (tb + 1) * P
                nc.vector.tensor_tensor(
                    out=d16[:, W - P : W], in0=d16[:, W - P : W], in1=mask_sb[:],
                    op=MIN,
                )

            def scan(tb, d16):
                # suf[s] = sum_{k>=s} d16[k]  (reverse cumsum, frozen at t-1)
                W = (tb + 1) * P
                suf = pwork.tile([P, T], F32, tag="suf")
                nc.vector.tensor_tensor_scan(
                    out=suf[:, W - 1 :: -1],
                    data0=d16[:, W - 1 :: -1],
                    data1=zeros16[:, :W],
                    initial=0.0,
                    op0=ADD,
                    op1=ADD,
                )
                return suf

            def exp_act(tb, suf):
                W = (tb + 1) * P
                nhl_c = nhl_sb[:, tb : tb + 1]
                E = pwork.tile([P, T], F16, tag="E")
                nc.scalar.activation(
                    out=E[:, :W], in_=suf[:, :W], func=Exp,
                    bias=ebias[:], scale=nhl_c
                )
                return E

            def tt1(tb, d16, E):
                # Wf = d16*E, all-fp16 SBUF -> DVE 2x mode
                W = (tb + 1) * P
                Wf = pwork.tile([P, T], F16, tag="Wf")
                nc.vector.tensor_tensor(
                    out=Wf[:, :W], in0=d16[:, :W], in1=E[:, :W], op=MULT
                )
                return Wf

            def transpose_wf(tb, Wf):
                # one DMA-transpose instruction: out [P, tb+1, P] holds the
                # per-128-block transposes of Wf stacked along the free dim
                W = (tb + 1) * P
                WfT = pwork.tile([P, NJ, P], F16, tag="WfT")
                nc.sync.dma_start_transpose(
                    out=WfT[:, : tb + 1, :], in_=Wf[:, :W]
                )
                return WfT

            def reduce_mm(tb, WfT):
                # den/num: [WfT_kb]^T @ [ones | y_kb], accumulated over kb
                for kb in range(tb + 1):
                    nc.tensor.matmul(
                        out=redps[:, 2 * tb : 2 * tb + 2],
                        lhsT=WfT[:, kb, :],
                        rhs=yok_sb[:, 2 * kb : 2 * kb + 2],
                        start=(kb == 0),
                        stop=(kb == tb),
                    )

            # software pipeline over tb:
            #   PE : matmul(i+2), reduce_mm(i-2)
            #   DVE: scan(i), tt1(i-1), mask(i+1)
            #   ACT: d16(i+1), E(i)
            #   DMA: transpose_wf(i-1)
            raws, d16s, Es, sufs, Wfs, WfTs = {}, {}, {}, {}, {}, {}
            raws[0] = matmul(0)
            d16s[0] = d16_copy(0, raws[0])
            mask(0, d16s[0])
            raws[1] = matmul(1)
            for i in range(NJ + 2):
                if i == 0 and b + 1 < NB_PER_CORE:
                    nxt = prep_dma(b + 1)
                if i < NJ:
                    sufs[i] = scan(i, d16s[i])
                    Es[i] = exp_act(i, sufs[i])
                if i + 2 <= NJ - 1:
                    raws[i + 2] = matmul(i + 2)
                if i + 1 <= NJ - 1:
                    d16s[i + 1] = d16_copy(i + 1, raws[i + 1])
                    mask(i + 1, d16s[i + 1])
                if 0 <= i - 1 < NJ:
                    Wfs[i - 1] = tt1(i - 1, d16s[i - 1], Es[i - 1])
                    WfTs[i - 1] = transpose_wf(i - 1, Wfs[i - 1])
                if 0 <= i - 2 < NJ:
                    reduce_mm(i - 2, WfTs[i - 2])

            # ---- finalize (den at even cols, num at odd cols) ----
            wse = psmall.tile([P, NJ], F32, tag="wse")
            nc.vector.tensor_scalar(
                out=wse[:], in0=redps[:, 0 : 2 * NJ : 2], scalar1=2e-6 * 1024.0,
                scalar2=None, op0=ADD
            )
            rcp = psmall.tile([P, NJ], F32, tag="rcp")
            nc.vector.reciprocal(out=rcp[:], in_=wse[:])
            yh = psmall.tile([P, NJ], F32, tag="yh")
            nc.vector.tensor_tensor(
                out=yh[:], in0=redps[:, 1 : 2 * NJ : 2], in1=rcp[:], op=MULT
            )
            yc = psmall.tile([P, NJ], F32, tag="yc")
            nc.vector.tensor_scalar(
                out=yc[:], in0=yh[:], scalar1=0.01, scalar2=0.99, op0=MAX, op1=MIN
            )
            nc.sync.dma_start(out=out[b], in_=yc[:])


def shard_inputs(y, problem_seq, embd_weight, lam_w, lam_b):
    """Build per-core input maps (host-side layout prep, not device time)."""
    B = y.shape[0]
    assert B == N_CORES * NB_PER_CORE
    seq = np.ascontiguousarray(problem_seq).astype(np.int64)
    yf = np.ascontiguousarray(y).astype(np.float32)
    emb = np.ascontiguousarray(embd_weight).astype(np.float32)
    lamw = np.asarray(lam_w, dtype=np.float32).reshape(P, 1)
    lamb = np.float32(np.asarray(lam_b).reshape(-1)[0])

    norm = np.linalg.norm(emb, axis=1, keepdims=True)
    xhat16 = (emb / norm).astype(np.float16)           # [V, H]
    nhl32 = (-0.5 * np.exp(emb @ lamw + lamb)).astype(np.float32)[:, 0]  # [V]

    colv, rowv = np.meshgrid(np.arange(P), np.arange(P))
    # min-mask on d16 = raw+1: pass below diagonal, clamp to 0 at/above
    mask16 = np.where(colv < rowv, np.float16(65504.0), np.float16(0.0)).astype(
        np.float16
    )

    in_maps = []
    for c in range(N_CORES):
        sl = slice(c * NB_PER_CORE, (c + 1) * NB_PER_CORE)
        seq_c = seq[sl]                                 # [NB, T]
        # xh[b, h, t] = xhat16[seq[b, t], h]
        xh = np.ascontiguousarray(
            xhat16[seq_c].transpose(0, 2, 1)            # [NB, H, T]
        )
        # nhl[b, p, tb] = -lam/2 of token tb*128+p
        nhl_c = np.ascontiguousarray(
            nhl32[seq_c].reshape(NB_PER_CORE, NJ, P).transpose(0, 2, 1)
        )
        # yok[b, k, 2*kb] = 1, yok[b, k, 2*kb+1] = y[b, kb*128+k]
        yok_c = np.zeros((NB_PER_CORE, P, 2 * NJ), np.float16)
        yok_c[:, :, 0::2] = 1.0
        yok_c[:, :, 1::2] = (
            yf[sl].reshape(NB_PER_CORE, NJ, P).transpose(0, 2, 1).astype(np.float16)
        )
        in_maps.append(
            {
                "xh": xh,
                "nhl": nhl_c,
                "yok": np.ascontiguousarray(yok_c),
                "mask16": mask16,
            }
        )
    return in_maps


def unshard_output(results):
    """results: list of 8 dicts with 'out' [4, 128, 8] -> yhat [32, 1024]."""
    parts = []
    for c in range(N_CORES):
        o = results[c]["out"]  # [NB, P, NJ]; yhat[b, j*128+p] = o[b, p, j]
        parts.append(o.transpose(0, 2, 1).reshape(NB_PER_CORE, T))
    return np.concatenate(parts, axis=0).astype(np.float32)


_NC_CACHE = None


def _get_program():
    global _NC_CACHE
    if _NC_CACHE is None:
        _NC_CACHE = build_program()
    return _NC_CACHE


def kernel(y, problem_seq, embd_weight, lam_w, lam_b, _trace=False, **trace_kwargs):
    from concourse.bass_utils import run_bass_kernel_spmd

    nc = _get_program()
    in_maps = shard_inputs(y, problem_seq, embd_weight, lam_w, lam_b)
    res = run_bass_kernel_spmd(
        nc, in_maps, core_ids=list(range(N_CORES)), trace=_trace, **trace_kwargs
    )
    outp = unshard_output(res.results)
    if _trace:
        return outp, res
    return outp


if __name__ == "__main__":
    rng = np.random.default_rng(0)
    y = rng.random((32, T), dtype=np.float32)
    seq = rng.integers(0, 50000, size=(32, T)).astype(np.int32)
    emb = rng.standard_normal((50000, P), dtype=np.float32)
    lw = (rng.standard_normal((P, 1), dtype=np.float32) / np.sqrt(P)).astype(np.float32)
    lb = (rng.standard_normal((1,), dtype=np.float32) * 0.01).astype(np.float32)
    outp = kernel(y, seq, emb, lw, lb)
    print("out", outp.shape, outp.dtype, outp[:2, :5])
